# revision 4
# baseline (speedup 1.0000x reference)
"""2-layer GraphSAGE (mean aggregation) on 8 trn2 NeuronCores — v2.

Changes vs v1 (the 2.46ms baseline):
  - 4 SWDGE queues. Edge-message gathers are split between two paths that
    drain concurrently on different queue pairs:
      * HBM path (queues 0-1): transpose=False row gathers from padded
        node-major DRAM copies of x / h (v1's scheme) — bound ~55 GB/s/core
        by HBM random-read.
      * SBUF path (queues 2-3): transpose=True gathers from an SBUF-resident
        wrapped token table (token i -> partition i%128, 256B stripe i//128).
        Output is feature-major [128f, E]; PE transposes (identity matmul)
        restore edge-major tiles, an ACT copy moves them PSUM->SBUF.
    The same SBUF table tile holds x during layer 1 and h during layer 2.
  - One-hot segment-sum matmul as v1, but one-hots are built in per-(block,
    stream) batches with a single stride-0-broadcast tensor_tensor per run.
  - h is stored p-major (row p*nblk+b = node b*128+p) so the post-AllGather
    SBUF table reload is a fully contiguous 12.8MB DMA.
  - dense phase in bf16 (weights/agg/own), PSUM accumulation in f32.
"""

import numpy as np
import ml_dtypes

import concourse.bacc as bacc
import concourse.mybir as mybir
import concourse.tile as tile
from concourse.bass import AP
from concourse.bass_utils import run_bass_kernel_spmd

P = 128
D = 64
F32 = mybir.dt.float32
BF16 = mybir.dt.bfloat16
I16 = mybir.dt.int16
U8 = mybir.dt.uint8
BF = ml_dtypes.bfloat16

N = 50000
NC = 8
N_OWN = N // NC                  # 6250
NBLK = -(-N_OWN // P)            # 49
NP_ = NBLK * P                   # 6272 padded nodes per core
NPALL = NP_ * NC                 # 50176
NTOK_X = -(-N // P) * P          # 50048 x-table tokens
NRANK_X = NTOK_X // P            # 391
NRANK_H = NPALL // P             # 392
SPLIT_SB = 31232                 # token-space region split (244 stripes)
SPLIT_HX = 5 * N_OWN             # 31250 x row-space split
SPLIT_HH = 5 * NP_               # 31360 h row-space split
SB_HI_OFF = (SPLIT_SB // P) * 256  # 62464 B

# streams: 0 = HBM-lo, 1 = HBM-hi, 2 = SBUF-lo, 3 = SBUF-hi
H_LO, H_HI, S_LO, S_HI = 0, 1, 2, 3


class Meta:
    pass


def _wrap16(v):
    assert v.shape[0] % 16 == 0
    return np.ascontiguousarray(v.reshape(-1, 16).T)


def preprocess(edge_index, sbuf8=3, chunk_h=16, chunk_s=16):
    """Partition edges into (core, block, stream) groups; build idx/aux
    tables. sbuf8/8 of edges go via the SBUF gather path."""
    src = np.asarray(edge_index[0], dtype=np.int64)
    dst = np.asarray(edge_index[1], dtype=np.int64)
    E = src.shape[0]

    cnt = np.bincount(dst, minlength=N).astype(np.float32)
    inv = (1.0 / np.maximum(cnt, 1.0)).astype(np.float32)

    core = dst // N_OWN
    dstl = dst - core * N_OWN
    blk = dstl // P
    inb = dstl - blk * P

    c_src = src // N_OWN
    l_src = src - c_src * N_OWN
    pos = c_src * NP_ + l_src                          # padded token id
    rowh = c_src * NP_ + (l_src % P) * NBLK + l_src // P  # h DRAM row (v2)

    path_sbuf = (src % 8) < sbuf8
    region = np.where(path_sbuf, src >= SPLIT_SB, src >= SPLIT_HX)
    stream = np.where(path_sbuf, 2, 0) + region

    key = (core * NBLK + blk) * 4 + stream
    ngroups = NC * NBLK * 4
    gcnt = np.bincount(key, minlength=ngroups).reshape(NC, NBLK, 4)
    # uniform (max over cores) tile counts per (block, stream)
    T = -(-gcnt.max(axis=0) // P)                      # [NBLK, 4]
    offs = np.zeros((4, NBLK + 1), np.int64)
    for s in range(4):
        offs[s, 1:] = np.cumsum(T[:, s])
    t_str = offs[:, -1]                                # tiles per stream
    t_off = np.concatenate([[0], np.cumsum(t_str)])    # global stream offset
    T_ALL = int(t_off[-1])

    order = np.argsort(key, kind="stable")
    gstart = np.concatenate([[0], np.cumsum(np.bincount(key, minlength=ngroups))])[:-1]
    rank = np.empty(E, dtype=np.int64)
    rank[order] = np.arange(E) - gstart[key[order]]

    gtile = t_off[stream] + offs[stream, blk] + rank // P  # global tile id
    slot = gtile * P + rank % P

    # per-layer idx values
    i1 = np.where(path_sbuf, src - np.where(region, SPLIT_SB, 0),
                  src - np.where(region, SPLIT_HX, 0))
    i2 = np.where(path_sbuf, pos - np.where(region, SPLIT_SB, 0),
                  rowh - np.where(region, SPLIT_HH, 0))
    assert i1.min() >= 0 and i1.max() < 32768, (i1.min(), i1.max())
    assert i2.min() >= 0 and i2.max() < 32768, (i2.min(), i2.max())

    meta = Meta()
    meta.T_ALL = T_ALL
    meta.t_off = t_off
    meta.offs = offs
    meta.sbuf8 = sbuf8
    meta.block_tiles = [
        [(s, int(t_off[s] + t)) for s in range(4)
         for t in range(int(offs[s, b]), int(offs[s, b + 1]))]
        for b in range(NBLK)
    ]

    meta.idx = []    # [128, T_ALL*8*2] int16 : layer1 | layer2
    meta.dstf = []   # [128, T_ALL] bf16 (-1 for empty slots)
    meta.invb = []   # [64, NP_] bf16
    for k in range(NC):
        m = core == k
        sl = slot[m]
        ia = np.zeros(T_ALL * P, np.int16)
        ib = np.zeros(T_ALL * P, np.int16)
        ia[sl] = i1[m]
        ib[sl] = i2[m]
        w = np.concatenate([_wrap16(ia), _wrap16(ib)], axis=1)
        meta.idx.append(np.ascontiguousarray(np.tile(w, (8, 1))))

        df = np.full(T_ALL * P, -1.0, np.float32)
        df[sl] = inb[m]
        meta.dstf.append(np.ascontiguousarray(
            df.reshape(T_ALL, P).T.astype(BF)))

        iv = np.ones(NP_, np.float32)
        iv[:N_OWN] = inv[k * N_OWN:(k + 1) * N_OWN]
        meta.invb.append(np.ascontiguousarray(
            np.tile(iv, (D, 1)).astype(BF)))

    # gather calls per path: (stream, t0_in_stream, ntiles, first_block)
    def chunks(s, chunk):
        out = []
        t0 = 0
        tot = int(t_str[s])
        while t0 < tot:
            nt = min(chunk, tot - t0)
            fb = int(np.searchsorted(offs[s], t0, side="right") - 1)
            out.append((s, t0, nt, fb))
            t0 += nt
        return out

    hcalls = chunks(H_LO, chunk_h) + chunks(H_HI, chunk_h)
    scalls = chunks(S_LO, chunk_s) + chunks(S_HI, chunk_s)
    hcalls.sort(key=lambda c: (c[3], c[0]))
    scalls.sort(key=lambda c: (c[3], c[0]))
    # merge by first_block for program order
    allcalls = [(0, c) for c in hcalls] + [(1, c) for c in scalls]
    allcalls.sort(key=lambda pc: (pc[1][3], pc[0], pc[1][0]))
    meta.calls = allcalls
    meta.chunk_h, meta.chunk_s = chunk_h, chunk_s
    return meta


GCOL = 512  # dense-phase group width (one PSUM bank)
TGRP = 8    # SBUF-path tiles per PSUM transpose group


def _bcast3(ap, inner_rep):
    """[128, L] AP -> [128, L, inner_rep] with stride-0 inner dim."""
    new = [list(d) for d in ap.ap] + [[0, inner_rep]]
    return AP(ap.tensor, ap.offset, new)


def _rep3(ap, mid_rep):
    """[128, C] AP -> [128, mid_rep, C] repeating the C cols mid_rep times."""
    new = [list(ap.ap[0]), [0, mid_rep], list(ap.ap[1])]
    return AP(ap.tensor, ap.offset, new)


def build_program(meta, one_core=False, reps=1,
                  parts=("gather", "agg", "dense", "store", "collective"),
                  sq=1, hq=3):
    ncores = 1 if one_core else NC
    nc = bacc.Bacc(
        "TRN2", target_bir_lowering=False, debug=False,
        num_devices=ncores, num_swdge_queues=4,
    )
    T_ALL = meta.T_ALL
    CH, CS = meta.chunk_h, meta.chunk_s

    xn_dr = nc.dram_tensor("xn", [N, P], BF16, kind="ExternalInput")
    xw_dr = nc.dram_tensor("xw", [P * NRANK_X, P], BF16, kind="ExternalInput")
    idx_dr = nc.dram_tensor("idx", list(meta.idx[0].shape), I16,
                            kind="ExternalInput")
    dstf_dr = nc.dram_tensor("dstf", [P, T_ALL], BF16, kind="ExternalInput")
    invb_dr = nc.dram_tensor("invb", [D, NP_], BF16, kind="ExternalInput")
    wl1_dr = nc.dram_tensor("wl1t", [D, D], BF16, kind="ExternalInput")
    wr1_dr = nc.dram_tensor("wr1t", [D, D], BF16, kind="ExternalInput")
    wl2_dr = nc.dram_tensor("wl2t", [D, D], BF16, kind="ExternalInput")
    wr2_dr = nc.dram_tensor("wr2t", [D, D], BF16, kind="ExternalInput")
    b1_dr = nc.dram_tensor("b1", [D, 1], F32, kind="ExternalInput")
    b2_dr = nc.dram_tensor("b2", [D, 1], F32, kind="ExternalInput")
    iota_dr = nc.dram_tensor("iota", [P, P], BF16, kind="ExternalInput")
    id_dr = nc.dram_tensor("ident", [D, D], BF16, kind="ExternalInput")
    id32_dr = nc.dram_tensor("ident32", [D, D], F32, kind="ExternalInput")
    xoT_dr = nc.dram_tensor("xoT", [D, NP_], BF16, kind="ExternalInput")
    out_dr = nc.dram_tensor("out", [NP_, D], F32, kind="ExternalOutput")

    with tile.TileContext(nc) as tc:
        with (
            tc.tile_pool(name="const", bufs=1) as cpool,
            tc.tile_pool(name="big", bufs=1) as bpool,
            tc.tile_pool(name="mh", bufs=3) as mhpool,
            tc.tile_pool(name="mt", bufs=3) as mtpool,
            tc.tile_pool(name="msb", bufs=6) as msbpool,
            tc.tile_pool(name="idxp", bufs=6) as ipool,
            tc.tile_pool(name="ohp", bufs=8) as ohpool,
            tc.tile_pool(name="grp", bufs=2) as gpool,
            tc.tile_pool(name="psT", bufs=2, space="PSUM") as psT,
            tc.tile_pool(name="psA", bufs=2, space="PSUM") as psA,
            tc.tile_pool(name="psZ", bufs=2, space="PSUM") as psZ,
            tc.tile_pool(name="psS", bufs=2, space="PSUM") as psS,
            tc.tile_pool(name="dram", bufs=1, space="DRAM") as dpool,
        ):
            def load(pool, dr, shape, name, dt=BF16):
                t = pool.tile(shape, dt, name=name, tag=name)
                nc.sync.dma_start(out=t, in_=dr.ap())
                return t

            iota_sb = load(cpool, iota_dr, [P, P], "iota_sb")
            ident_sb = load(cpool, id_dr, [D, D], "ident_sb")
            ident32_sb = load(cpool, id32_dr, [D, D], "ident32_sb", dt=F32)
            wl1_sb = load(cpool, wl1_dr, [D, D], "wl1_sb")
            wr1_sb = load(cpool, wr1_dr, [D, D], "wr1_sb")
            wl2_sb = load(cpool, wl2_dr, [D, D], "wl2_sb")
            wr2_sb = load(cpool, wr2_dr, [D, D], "wr2_sb")
            b1_sb = load(cpool, b1_dr, [D, 1], "b1_sb", dt=F32)
            b2_sb = load(cpool, b2_dr, [D, 1], "b2_sb", dt=F32)
            dstf_sb = load(bpool, dstf_dr, [P, T_ALL], "dstf_sb")
            invb_sb = load(bpool, invb_dr, [D, NP_], "invb_sb")
            xoT_sb = load(bpool, xoT_dr, [D, NP_], "xoT_sb")
            hT_sb = bpool.tile([D, NP_], BF16, name="hT_sb")
            nodeh_sb = bpool.tile([P, NBLK * P], BF16, name="nodeh_sb")
            nodeo_sb = bpool.tile([P, NBLK * D], F32, name="nodeo_sb")
            nc.vector.memset(nodeh_sb, 0.0)

            # wrapped token table (x for layer 1, h for layer 2)
            use_tbl = int(meta.t_off[4]) > int(meta.t_off[2])
            if use_tbl:
                tbl = bpool.tile([P, NRANK_H * 256], U8, name="tbl")
                tblh = tbl.bitcast(BF16)

            for rep in range(reps):
              h_chunk = dpool.tile([NP_, P], BF16, name=f"h_chunk_{rep}",
                                   tag=f"hc{rep}")
              h_full = dpool.tile([NPALL, P], BF16, name=f"h_full_{rep}",
                                  tag=f"hf{rep}", addr_space="Shared")
              if use_tbl:
                  # load x into the token table (contiguous)
                  nc.sync.dma_start(
                      out=tblh[:, :NRANK_X * P].rearrange("p (t f) -> p t f",
                                                          f=P),
                      in_=xw_dr.ap().rearrange("(p t) f -> p t f", p=P),
                  )
              for layer in range(2):
                ioff = layer * T_ALL * 8
                if layer == 0:
                    hbm_lo = xn_dr.ap()[0:SPLIT_HX, :]
                    hbm_hi = xn_dr.ap()[SPLIT_HX:N, :]
                    wl_sb, wr_sb, bb_sb = wl1_sb, wr1_sb, b1_sb
                    own_sb = xoT_sb
                    func = mybir.ActivationFunctionType.Tanh
                else:
                    hbm_lo = h_full[0:SPLIT_HH, :]
                    hbm_hi = h_full[SPLIT_HH:NPALL, :]
                    wl_sb, wr_sb, bb_sb = wl2_sb, wr2_sb, b2_sb
                    own_sb = hT_sb
                    func = mybir.ActivationFunctionType.Identity

                # ---- gathers ----
                tsrc = {}
                qh, qs = 0, 0
                for ci, (pth, (s, t0, nt, _fb)) in enumerate(meta.calls):
                    if "gather" not in parts:
                        break
                    it = ipool.tile([P, max(CH, CS) * 8], I16, tag="idx",
                                    name=f"i_{layer}_{ci}")
                    cols = nt * 8
                    coff = ioff + (int(meta.t_off[s]) + t0) * 8
                    nc.sync.dma_start(out=it[:, :cols],
                                      in_=idx_dr.ap()[:, coff:coff + cols])
                    gbase = int(meta.t_off[s]) + t0
                    if pth == 0:
                        mt = mhpool.tile([P, CH, P], BF16, tag="mh",
                                         name=f"mh_{layer}_{ci}")
                        nc.gpsimd.dma_gather(
                            mt[:, :nt, :],
                            hbm_lo if s == H_LO else hbm_hi,
                            it[:, :cols],
                            num_idxs=nt * P, num_idxs_reg=nt * P,
                            elem_size=P, single_packet=False,
                            queue_num=qh % hq,
                        )
                        qh += 1
                        for j in range(nt):
                            tsrc[gbase + j] = (mt, j, 0)
                    else:
                        mtT = mtpool.tile([P, CS * P], BF16, tag="mt",
                                          name=f"mt_{layer}_{ci}")
                        src_ap = (tbl[:, 0:SB_HI_OFF] if s == S_LO
                                  else tbl[:, SB_HI_OFF:])
                        nc.gpsimd.dma_gather(
                            mtT[:, 0:nt * P].rearrange(
                                "p (a b) -> p a b", a=1),
                            src_ap, it[:, :cols],
                            num_idxs=nt * P, num_idxs_reg=nt * P,
                            elem_size=P, transpose=True,
                            single_packet=False, queue_num=2 + qs % sq,
                            sbuf_tokens_per_rank=P,
                            sbuf_free_dim_per_rank=256,
                            sbuf_free_dim_pad_per_rank=0,
                            sbuf_byte_offset=0,
                        )
                        qs += 1
                        # transpose back to edge-major in groups of TGRP
                        for g0 in range(0, nt, TGRP):
                            gn = min(TGRP, nt - g0)
                            pt = psT.tile([P, TGRP * D], BF16, tag="pt",
                                          name=f"pt_{layer}_{ci}_{g0}")
                            for j in range(gn):
                                nc.tensor.transpose(
                                    out=pt[:, j * D:(j + 1) * D],
                                    in_=mtT[0:D,
                                            (g0 + j) * P:(g0 + j + 1) * P],
                                    identity=ident_sb,
                                )
                            ms = msbpool.tile([P, TGRP * D], BF16, tag="ms",
                                              name=f"ms_{layer}_{ci}_{g0}")
                            nc.scalar.copy(out=ms[:, :gn * D],
                                           in_=pt[:, :gn * D])
                            for j in range(gn):
                                tsrc[gbase + g0 + j] = (ms, j, 1)

                # ---- aggregation + dense ----
                BPG = GCOL // P
                ngrp = -(-NBLK // BPG)
                for g in range(ngrp if "agg" in parts else 0):
                    b0 = g * BPG
                    nb = min(BPG, NBLK - b0)
                    w = nb * P
                    aggT = gpool.tile([D, GCOL], BF16, tag="aggT",
                                      name=f"agg_{rep}_{layer}_{g}")
                    psg = psA.tile([D, GCOL], F32, tag="agg",
                                   name=f"ps_{layer}_{g}")
                    for bi in range(nb):
                        b = b0 + bi
                        tl = meta.block_tiles[b]
                        if not tl:
                            nc.vector.memset(
                                aggT[:, bi * P:(bi + 1) * P], 0.0)
                            continue
                        ps = psg[:, bi * P:(bi + 1) * P]
                        # one-hot runs per stream within the block
                        runs = []
                        for s in range(4):
                            gts = [gt for (ss, gt) in tl if ss == s]
                            while len(gts) > 4:
                                runs.append(gts[:4])
                                gts = gts[4:]
                            if gts:
                                runs.append(gts)
                        ohmap = {}
                        for gts in runs:
                            ln = len(gts)
                            oh = ohpool.tile([P, ln * P], BF16, tag="oh",
                                             name=f"oh_{layer}_{b}_{gts[0]}")
                            nc.vector.tensor_tensor(
                                out=oh.rearrange("p (l c) -> p l c", c=P),
                                in0=_rep3(iota_sb[:, 0:P], ln),
                                in1=_bcast3(dstf_sb[:, gts[0]:gts[0] + ln], P),
                                op=mybir.AluOpType.is_equal,
                            )
                            for j, gt in enumerate(gts):
                                ohmap[gt] = (oh, j)
                        nt_tot = len(tl)
                        for j, (s, gt) in enumerate(tl):
                            mtile, lt, kind = tsrc[gt]
                            lhsT = (mtile[:, lt, 0:D] if kind == 0
                                    else mtile[:, lt * D:(lt + 1) * D])
                            oh, oj = ohmap[gt]
                            nc.tensor.matmul(
                                ps, lhsT=lhsT,
                                rhs=oh[:, oj * P:(oj + 1) * P],
                                start=(j == 0), stop=(j == nt_tot - 1),
                            )
                        nc.vector.tensor_tensor(
                            out=aggT[:, bi * P:(bi + 1) * P], in0=ps,
                            in1=invb_sb[:, b * P:(b + 1) * P],
                            op=mybir.AluOpType.mult,
                        )
                    if "dense" not in parts:
                        continue
                    zp = psZ.tile([D, GCOL], F32, tag="z",
                                  name=f"z_{layer}_{g}")
                    nc.tensor.matmul(zp[:, :w], lhsT=wl_sb, rhs=aggT[:, :w],
                                     start=True, stop=False)
                    nc.tensor.matmul(zp[:, :w], lhsT=wr_sb,
                                     rhs=own_sb[:, b0 * P:b0 * P + w],
                                     start=False, stop=True)
                    if layer == 0:
                        outT = hT_sb
                        nc.scalar.activation(out=hT_sb[:, b0 * P:b0 * P + w],
                                             in_=zp[:, :w], func=func,
                                             bias=bb_sb[:, 0:1], scale=1.0)
                    else:
                        outT = gpool.tile([D, GCOL], F32, tag="outT",
                                          name=f"oT_{rep}_{g}")
                        nc.scalar.activation(out=outT[:, :w], in_=zp[:, :w],
                                             func=func, bias=bb_sb[:, 0:1],
                                             scale=1.0)
                    if "store" not in parts:
                        continue
                    for bi in range(nb):
                        b = b0 + bi
                        tp = psS.tile([P, D],
                                      BF16 if layer == 0 else F32,
                                      tag="tr", name=f"tp_{layer}_{b}")
                        sl = (slice(b * P, b * P + P) if layer == 0
                              else slice(bi * P, bi * P + P))
                        nc.tensor.transpose(
                            out=tp, in_=outT[:, sl],
                            identity=ident_sb if layer == 0 else ident32_sb)
                        if layer == 0:
                            nc.scalar.copy(out=nodeh_sb[:, b * P:b * P + D],
                                           in_=tp)
                        else:
                            nc.scalar.copy(out=nodeo_sb[:, b * D:(b + 1) * D],
                                           in_=tp)

                if layer == 0 and "store" in parts:
                    # p-major store: h_chunk row p*NBLK+b = node b*128+p
                    nc.sync.dma_start(
                        out=h_chunk.rearrange("(p b) f -> p b f", b=NBLK),
                        in_=nodeh_sb.rearrange("p (b f) -> p b f", f=P),
                    )
                elif layer == 1 and "store" in parts:
                    nc.sync.dma_start(
                        out=out_dr.ap().rearrange("(b p) f -> p b f", p=P),
                        in_=nodeo_sb.rearrange("p (b f) -> p b f", f=D),
                    )
                if layer == 0 and "collective" in parts:
                    if one_core:
                        nc.sync.dma_start(out=h_full[0:NP_, :], in_=h_chunk)
                    else:
                        nc.gpsimd.collective_compute(
                            "AllGather",
                            mybir.AluOpType.bypass,
                            replica_groups=[list(range(NC))],
                            ins=[h_chunk.opt()],
                            outs=[h_full.opt()],
                        )
                    if use_tbl:
                        # reload the token table with h (contiguous)
                        nc.sync.dma_start(
                            out=tblh.rearrange("p (c b f) -> p c b f",
                                               c=NC, f=P),
                            in_=h_full.rearrange("(c p b) f -> p c b f",
                                                 p=P, b=NBLK),
                        )

    nc.compile()
    return nc


def make_in_maps(meta, x, W_l1, b_l1, W_r1, W_l2, b_l2, W_r2):
    x = np.ascontiguousarray(np.asarray(x, dtype=np.float32))
    xb = x.astype(BF)
    xn = np.zeros((N, P), BF)
    xn[:, :D] = xb
    # wrapped p-major copy: row p*NRANK_X + t = node t*128+p (padded)
    xpad = np.zeros((NTOK_X, P), BF)
    xpad[:N, :D] = xb
    xw = np.ascontiguousarray(
        xpad.reshape(NRANK_X, P, P).transpose(1, 0, 2).reshape(P * NRANK_X, P))
    iota = np.tile(np.arange(P, dtype=np.float32), (P, 1)).astype(BF)
    ident = np.eye(D, dtype=np.float32).astype(BF)
    common = {
        "xn": xn,
        "xw": xw,
        "wl1t": np.ascontiguousarray(np.asarray(W_l1, np.float32).T).astype(BF),
        "wr1t": np.ascontiguousarray(np.asarray(W_r1, np.float32).T).astype(BF),
        "wl2t": np.ascontiguousarray(np.asarray(W_l2, np.float32).T).astype(BF),
        "wr2t": np.ascontiguousarray(np.asarray(W_r2, np.float32).T).astype(BF),
        "b1": np.asarray(b_l1, np.float32).reshape(D, 1).copy(),
        "b2": np.asarray(b_l2, np.float32).reshape(D, 1).copy(),
        "iota": iota,
        "ident": ident,
        "ident32": np.eye(D, dtype=np.float32),
    }
    in_maps = []
    for k in range(NC):
        xo = xb[k * N_OWN:(k + 1) * N_OWN]
        xoT = np.zeros((D, NP_), BF)
        xoT[:, :N_OWN] = xo.T
        in_maps.append(dict(common, xoT=xoT, idx=meta.idx[k],
                            dstf=meta.dstf[k], invb=meta.invb[k]))
    return in_maps


_CACHE = {}
_LAST_RES = None


def kernel(x, edge_index, W_l1, b_l1, W_r1, W_l2, b_l2, W_r2):
    edge_index = np.asarray(edge_index)
    x = np.asarray(x)
    key = hash(edge_index.tobytes())
    if key in _CACHE:
        meta, nc = _CACHE[key]
    else:
        meta = preprocess(edge_index)
        nc = build_program(meta)
        _CACHE[key] = (meta, nc)
    in_maps = make_in_maps(meta, x, W_l1, b_l1, W_r1, W_l2, b_l2, W_r2)
    res = run_bass_kernel_spmd(nc, in_maps, core_ids=list(range(NC)))
    global _LAST_RES
    _LAST_RES = res
    out = np.concatenate(
        [res.results[k]["out"][:N_OWN] for k in range(NC)], axis=0
    )
    return out.astype(np.float32)


# revision 5
# speedup vs baseline: 1.0662x; 1.0662x over previous
"""2-layer GraphSAGE (mean aggregation) on 8 trn2 NeuronCores — v2.

Changes vs v1 (the 2.46ms baseline):
  - 4 SWDGE queues. Edge-message gathers are split between two paths that
    drain concurrently on different queue pairs:
      * HBM path (queues 0-1): transpose=False row gathers from padded
        node-major DRAM copies of x / h (v1's scheme) — bound ~55 GB/s/core
        by HBM random-read.
      * SBUF path (queues 2-3): transpose=True gathers from an SBUF-resident
        wrapped token table (token i -> partition i%128, 256B stripe i//128).
        Output is feature-major [128f, E]; PE transposes (identity matmul)
        restore edge-major tiles, an ACT copy moves them PSUM->SBUF.
    The same SBUF table tile holds x during layer 1 and h during layer 2.
  - One-hot segment-sum matmul as v1, but one-hots are built in per-(block,
    stream) batches with a single stride-0-broadcast tensor_tensor per run.
  - h is stored p-major (row p*nblk+b = node b*128+p) so the post-AllGather
    SBUF table reload is a fully contiguous 12.8MB DMA.
  - dense phase in bf16 (weights/agg/own), PSUM accumulation in f32.
"""

import numpy as np
import ml_dtypes

import concourse.bacc as bacc
import concourse.mybir as mybir
import concourse.tile as tile
from concourse.bass import AP
from concourse.bass_utils import run_bass_kernel_spmd

P = 128
D = 64
F32 = mybir.dt.float32
BF16 = mybir.dt.bfloat16
I16 = mybir.dt.int16
U8 = mybir.dt.uint8
BF = ml_dtypes.bfloat16

N = 50000
NC = 8
N_OWN = N // NC                  # 6250
NBLK = -(-N_OWN // P)            # 49
NP_ = NBLK * P                   # 6272 padded nodes per core
NPALL = NP_ * NC                 # 50176
NTOK_X = -(-N // P) * P          # 50048 x-table tokens
NRANK_X = NTOK_X // P            # 391
NRANK_H = NPALL // P             # 392
SPLIT_SB = 31232                 # token-space region split (244 stripes)
SPLIT_HX = 5 * N_OWN             # 31250 x row-space split
SPLIT_HH = 5 * NP_               # 31360 h row-space split
SB_HI_OFF = (SPLIT_SB // P) * 256  # 62464 B

# streams: 0 = HBM-lo, 1 = HBM-hi, 2 = SBUF-lo, 3 = SBUF-hi
H_LO, H_HI, S_LO, S_HI = 0, 1, 2, 3


class Meta:
    pass


def _wrap16(v):
    assert v.shape[0] % 16 == 0
    return np.ascontiguousarray(v.reshape(-1, 16).T)


def preprocess(edge_index, sbuf8=3, chunk_h=16, chunk_s=16):
    """Partition edges into (core, block, stream) groups; build idx/aux
    tables. sbuf8/8 of edges go via the SBUF gather path."""
    src = np.asarray(edge_index[0], dtype=np.int64)
    dst = np.asarray(edge_index[1], dtype=np.int64)
    E = src.shape[0]

    cnt = np.bincount(dst, minlength=N).astype(np.float32)
    inv = (1.0 / np.maximum(cnt, 1.0)).astype(np.float32)

    core = dst // N_OWN
    dstl = dst - core * N_OWN
    blk = dstl // P
    inb = dstl - blk * P

    c_src = src // N_OWN
    l_src = src - c_src * N_OWN
    pos = c_src * NP_ + l_src                          # padded token id
    rowh = c_src * NP_ + (l_src % P) * NBLK + l_src // P  # h DRAM row (v2)

    path_sbuf = (src % 8) < sbuf8
    region = np.where(path_sbuf, src >= SPLIT_SB, src >= SPLIT_HX)
    stream = np.where(path_sbuf, 2, 0) + region

    key = (core * NBLK + blk) * 4 + stream
    ngroups = NC * NBLK * 4
    gcnt = np.bincount(key, minlength=ngroups).reshape(NC, NBLK, 4)
    # uniform (max over cores) tile counts per (block, stream)
    T = -(-gcnt.max(axis=0) // P)                      # [NBLK, 4]
    offs = np.zeros((4, NBLK + 1), np.int64)
    for s in range(4):
        offs[s, 1:] = np.cumsum(T[:, s])
    t_str = offs[:, -1]                                # tiles per stream
    t_off = np.concatenate([[0], np.cumsum(t_str)])    # global stream offset
    T_ALL = int(t_off[-1])

    order = np.argsort(key, kind="stable")
    gstart = np.concatenate([[0], np.cumsum(np.bincount(key, minlength=ngroups))])[:-1]
    rank = np.empty(E, dtype=np.int64)
    rank[order] = np.arange(E) - gstart[key[order]]

    gtile = t_off[stream] + offs[stream, blk] + rank // P  # global tile id
    slot = gtile * P + rank % P

    # per-layer idx values
    i1 = np.where(path_sbuf, src - np.where(region, SPLIT_SB, 0),
                  src - np.where(region, SPLIT_HX, 0))
    i2 = np.where(path_sbuf, pos - np.where(region, SPLIT_SB, 0),
                  rowh - np.where(region, SPLIT_HH, 0))
    assert i1.min() >= 0 and i1.max() < 32768, (i1.min(), i1.max())
    assert i2.min() >= 0 and i2.max() < 32768, (i2.min(), i2.max())

    meta = Meta()
    meta.T_ALL = T_ALL
    meta.t_off = t_off
    meta.offs = offs
    meta.sbuf8 = sbuf8
    meta.block_tiles = [
        [(s, int(t_off[s] + t)) for s in range(4)
         for t in range(int(offs[s, b]), int(offs[s, b + 1]))]
        for b in range(NBLK)
    ]

    meta.idx = []    # [128, T_ALL*8*2] int16 : layer1 | layer2
    meta.dstf = []   # [128, T_ALL] bf16 (-1 for empty slots)
    meta.invb = []   # [64, NP_] bf16
    for k in range(NC):
        m = core == k
        sl = slot[m]
        ia = np.zeros(T_ALL * P, np.int16)
        ib = np.zeros(T_ALL * P, np.int16)
        ia[sl] = i1[m]
        ib[sl] = i2[m]
        w = np.concatenate([_wrap16(ia), _wrap16(ib)], axis=1)
        meta.idx.append(np.ascontiguousarray(np.tile(w, (8, 1))))

        df = np.full(T_ALL * P, -1.0, np.float32)
        df[sl] = inb[m]
        meta.dstf.append(np.ascontiguousarray(
            df.reshape(T_ALL, P).T.astype(BF)))

        iv = np.ones(NP_, np.float32)
        iv[:N_OWN] = inv[k * N_OWN:(k + 1) * N_OWN]
        meta.invb.append(np.ascontiguousarray(
            np.tile(iv, (D, 1)).astype(BF)))

    # gather calls per path: (stream, t0_in_stream, ntiles, first_block)
    def chunks(s, chunk):
        out = []
        t0 = 0
        tot = int(t_str[s])
        while t0 < tot:
            nt = min(chunk, tot - t0)
            fb = int(np.searchsorted(offs[s], t0, side="right") - 1)
            out.append((s, t0, nt, fb))
            t0 += nt
        return out

    hcalls = chunks(H_LO, chunk_h) + chunks(H_HI, chunk_h)
    scalls = chunks(S_LO, chunk_s) + chunks(S_HI, chunk_s)
    hcalls.sort(key=lambda c: (c[3], c[0]))
    scalls.sort(key=lambda c: (c[3], c[0]))
    # merge by first_block for program order
    allcalls = [(0, c) for c in hcalls] + [(1, c) for c in scalls]
    allcalls.sort(key=lambda pc: (pc[1][3], pc[0], pc[1][0]))
    meta.calls = allcalls
    meta.chunk_h, meta.chunk_s = chunk_h, chunk_s
    return meta


GCOL = 512  # dense-phase group width (one PSUM bank)
TGRP = 8    # SBUF-path tiles per PSUM transpose group


def _bcast3(ap, inner_rep):
    """[128, L] AP -> [128, L, inner_rep] with stride-0 inner dim."""
    new = [list(d) for d in ap.ap] + [[0, inner_rep]]
    return AP(ap.tensor, ap.offset, new)


def _rep3(ap, mid_rep):
    """[128, C] AP -> [128, mid_rep, C] repeating the C cols mid_rep times."""
    new = [list(ap.ap[0]), [0, mid_rep], list(ap.ap[1])]
    return AP(ap.tensor, ap.offset, new)


def build_program(meta, one_core=False, reps=1,
                  parts=("gather", "agg", "dense", "store", "collective"),
                  sq=1, hq=3):
    ncores = 1 if one_core else NC
    nc = bacc.Bacc(
        "TRN2", target_bir_lowering=False, debug=False,
        num_devices=ncores, num_swdge_queues=4,
    )
    T_ALL = meta.T_ALL
    CH, CS = meta.chunk_h, meta.chunk_s

    xn_dr = nc.dram_tensor("xn", [N, P], BF16, kind="ExternalInput")
    xw_dr = nc.dram_tensor("xw", [P * NRANK_X, P], BF16, kind="ExternalInput")
    idx_dr = nc.dram_tensor("idx", list(meta.idx[0].shape), I16,
                            kind="ExternalInput")
    dstf_dr = nc.dram_tensor("dstf", [P, T_ALL], BF16, kind="ExternalInput")
    invb_dr = nc.dram_tensor("invb", [D, NP_], BF16, kind="ExternalInput")
    wl1_dr = nc.dram_tensor("wl1t", [D, D], BF16, kind="ExternalInput")
    wr1_dr = nc.dram_tensor("wr1t", [D, D], BF16, kind="ExternalInput")
    wl2_dr = nc.dram_tensor("wl2t", [D, D], BF16, kind="ExternalInput")
    wr2_dr = nc.dram_tensor("wr2t", [D, D], BF16, kind="ExternalInput")
    b1_dr = nc.dram_tensor("b1", [D, 1], F32, kind="ExternalInput")
    b2_dr = nc.dram_tensor("b2", [D, 1], F32, kind="ExternalInput")
    iota_dr = nc.dram_tensor("iota", [P, P], BF16, kind="ExternalInput")
    id_dr = nc.dram_tensor("ident", [D, D], BF16, kind="ExternalInput")
    id32_dr = nc.dram_tensor("ident32", [D, D], F32, kind="ExternalInput")
    xoT_dr = nc.dram_tensor("xoT", [D, NP_], BF16, kind="ExternalInput")
    out_dr = nc.dram_tensor("out", [NP_, D], F32, kind="ExternalOutput")

    with tile.TileContext(nc) as tc:
        with (
            tc.tile_pool(name="const", bufs=1) as cpool,
            tc.tile_pool(name="big", bufs=1) as bpool,
            tc.tile_pool(name="mh", bufs=3) as mhpool,
            tc.tile_pool(name="mt", bufs=3) as mtpool,
            tc.tile_pool(name="msb", bufs=6) as msbpool,
            tc.tile_pool(name="idxp", bufs=6) as ipool,
            tc.tile_pool(name="ohp", bufs=8) as ohpool,
            tc.tile_pool(name="grp", bufs=2) as gpool,
            tc.tile_pool(name="psT", bufs=2, space="PSUM") as psT,
            tc.tile_pool(name="psA", bufs=2, space="PSUM") as psA,
            tc.tile_pool(name="psZ", bufs=2, space="PSUM") as psZ,
            tc.tile_pool(name="psS", bufs=2, space="PSUM") as psS,
            tc.tile_pool(name="dram", bufs=1, space="DRAM") as dpool,
        ):
            def load(pool, dr, shape, name, dt=BF16):
                t = pool.tile(shape, dt, name=name, tag=name)
                nc.sync.dma_start(out=t, in_=dr.ap())
                return t

            iota_sb = load(cpool, iota_dr, [P, P], "iota_sb")
            ident_sb = load(cpool, id_dr, [D, D], "ident_sb")
            ident32_sb = load(cpool, id32_dr, [D, D], "ident32_sb", dt=F32)
            wl1_sb = load(cpool, wl1_dr, [D, D], "wl1_sb")
            wr1_sb = load(cpool, wr1_dr, [D, D], "wr1_sb")
            wl2_sb = load(cpool, wl2_dr, [D, D], "wl2_sb")
            wr2_sb = load(cpool, wr2_dr, [D, D], "wr2_sb")
            b1_sb = load(cpool, b1_dr, [D, 1], "b1_sb", dt=F32)
            b2_sb = load(cpool, b2_dr, [D, 1], "b2_sb", dt=F32)
            dstf_sb = load(bpool, dstf_dr, [P, T_ALL], "dstf_sb")
            invb_sb = load(bpool, invb_dr, [D, NP_], "invb_sb")
            xoT_sb = load(bpool, xoT_dr, [D, NP_], "xoT_sb")
            hT_sb = bpool.tile([D, NP_], BF16, name="hT_sb")
            nodeh_sb = bpool.tile([P, NBLK * P], BF16, name="nodeh_sb")
            nodeo_sb = bpool.tile([P, NBLK * D], F32, name="nodeo_sb")
            nc.vector.memset(nodeh_sb, 0.0)

            # wrapped token table (x for layer 1, h for layer 2)
            use_tbl = int(meta.t_off[4]) > int(meta.t_off[2])
            if use_tbl:
                tbl = bpool.tile([P, NRANK_H * 256], U8, name="tbl")
                tblh = tbl.bitcast(BF16)

            for rep in range(reps):
              h_chunk = dpool.tile([NP_, P], BF16, name=f"h_chunk_{rep}",
                                   tag=f"hc{rep}")
              h_full = dpool.tile([NPALL, P], BF16, name=f"h_full_{rep}",
                                  tag=f"hf{rep}", addr_space="Shared")
              if use_tbl:
                  # load x into the token table (contiguous)
                  nc.sync.dma_start(
                      out=tblh[:, :NRANK_X * P].rearrange("p (t f) -> p t f",
                                                          f=P),
                      in_=xw_dr.ap().rearrange("(p t) f -> p t f", p=P),
                  )
              for layer in range(2):
                ioff = layer * T_ALL * 8
                if layer == 0:
                    hbm_lo = xn_dr.ap()[0:SPLIT_HX, :]
                    hbm_hi = xn_dr.ap()[SPLIT_HX:N, :]
                    wl_sb, wr_sb, bb_sb = wl1_sb, wr1_sb, b1_sb
                    own_sb = xoT_sb
                    func = mybir.ActivationFunctionType.Tanh
                else:
                    hbm_lo = h_full[0:SPLIT_HH, :]
                    hbm_hi = h_full[SPLIT_HH:NPALL, :]
                    wl_sb, wr_sb, bb_sb = wl2_sb, wr2_sb, b2_sb
                    own_sb = hT_sb
                    func = mybir.ActivationFunctionType.Identity

                # ---- gathers ----
                tsrc = {}
                qh, qs = 0, 0
                hqueues = [0, 1, 3][:hq]
                for ci, (pth, (s, t0, nt, _fb)) in enumerate(meta.calls):
                    if "gather" not in parts:
                        break
                    it = ipool.tile([P, max(CH, CS) * 8], I16, tag="idx",
                                    name=f"i_{layer}_{ci}")
                    cols = nt * 8
                    coff = ioff + (int(meta.t_off[s]) + t0) * 8
                    nc.sync.dma_start(out=it[:, :cols],
                                      in_=idx_dr.ap()[:, coff:coff + cols])
                    gbase = int(meta.t_off[s]) + t0
                    if pth == 0:
                        mt = mhpool.tile([P, CH, P], BF16, tag="mh",
                                         name=f"mh_{layer}_{ci}")
                        nc.gpsimd.dma_gather(
                            mt[:, :nt, :],
                            hbm_lo if s == H_LO else hbm_hi,
                            it[:, :cols],
                            num_idxs=nt * P, num_idxs_reg=nt * P,
                            elem_size=P, single_packet=False,
                            queue_num=hqueues[qh % len(hqueues)],
                        )
                        qh += 1
                        for j in range(nt):
                            tsrc[gbase + j] = (mt, j, 0)
                    else:
                        mtT = mtpool.tile([P, CS * P], BF16, tag="mt",
                                          name=f"mt_{layer}_{ci}")
                        src_ap = (tbl[:, 0:SB_HI_OFF] if s == S_LO
                                  else tbl[:, SB_HI_OFF:])
                        nc.gpsimd.dma_gather(
                            mtT[:, 0:nt * P].rearrange(
                                "p (a b) -> p a b", a=1),
                            src_ap, it[:, :cols],
                            num_idxs=nt * P, num_idxs_reg=nt * P,
                            elem_size=P, transpose=True,
                            single_packet=False, queue_num=2 + qs % sq,
                            sbuf_tokens_per_rank=P,
                            sbuf_free_dim_per_rank=256,
                            sbuf_free_dim_pad_per_rank=0,
                            sbuf_byte_offset=0,
                        )
                        qs += 1
                        # transpose back to edge-major in groups of TGRP
                        for g0 in range(0, nt, TGRP):
                            gn = min(TGRP, nt - g0)
                            pt = psT.tile([P, TGRP * D], BF16, tag="pt",
                                          name=f"pt_{layer}_{ci}_{g0}")
                            for j in range(gn):
                                nc.tensor.transpose(
                                    out=pt[:, j * D:(j + 1) * D],
                                    in_=mtT[0:D,
                                            (g0 + j) * P:(g0 + j + 1) * P],
                                    identity=ident_sb,
                                )
                            ms = msbpool.tile([P, TGRP * D], BF16, tag="ms",
                                              name=f"ms_{layer}_{ci}_{g0}")
                            nc.scalar.copy(out=ms[:, :gn * D],
                                           in_=pt[:, :gn * D])
                            for j in range(gn):
                                tsrc[gbase + g0 + j] = (ms, j, 1)

                # ---- aggregation + dense ----
                BPG = GCOL // P
                ngrp = -(-NBLK // BPG)
                for g in range(ngrp if "agg" in parts else 0):
                    b0 = g * BPG
                    nb = min(BPG, NBLK - b0)
                    w = nb * P
                    aggT = gpool.tile([D, GCOL], BF16, tag="aggT",
                                      name=f"agg_{rep}_{layer}_{g}")
                    psg = psA.tile([D, GCOL], F32, tag="agg",
                                   name=f"ps_{layer}_{g}")
                    for bi in range(nb):
                        b = b0 + bi
                        tl = meta.block_tiles[b]
                        if not tl:
                            nc.vector.memset(
                                aggT[:, bi * P:(bi + 1) * P], 0.0)
                            continue
                        ps = psg[:, bi * P:(bi + 1) * P]
                        # one-hot runs per stream within the block
                        runs = []
                        for s in range(4):
                            gts = [gt for (ss, gt) in tl if ss == s]
                            while len(gts) > 4:
                                runs.append(gts[:4])
                                gts = gts[4:]
                            if gts:
                                runs.append(gts)
                        ohmap = {}
                        for gts in runs:
                            ln = len(gts)
                            oh = ohpool.tile([P, ln * P], BF16, tag="oh",
                                             name=f"oh_{layer}_{b}_{gts[0]}")
                            nc.vector.tensor_tensor(
                                out=oh.rearrange("p (l c) -> p l c", c=P),
                                in0=_rep3(iota_sb[:, 0:P], ln),
                                in1=_bcast3(dstf_sb[:, gts[0]:gts[0] + ln], P),
                                op=mybir.AluOpType.is_equal,
                            )
                            for j, gt in enumerate(gts):
                                ohmap[gt] = (oh, j)
                        nt_tot = len(tl)
                        for j, (s, gt) in enumerate(tl):
                            mtile, lt, kind = tsrc[gt]
                            lhsT = (mtile[:, lt, 0:D] if kind == 0
                                    else mtile[:, lt * D:(lt + 1) * D])
                            oh, oj = ohmap[gt]
                            nc.tensor.matmul(
                                ps, lhsT=lhsT,
                                rhs=oh[:, oj * P:(oj + 1) * P],
                                start=(j == 0), stop=(j == nt_tot - 1),
                            )
                        nc.vector.tensor_tensor(
                            out=aggT[:, bi * P:(bi + 1) * P], in0=ps,
                            in1=invb_sb[:, b * P:(b + 1) * P],
                            op=mybir.AluOpType.mult,
                        )
                    if "dense" not in parts:
                        continue
                    zp = psZ.tile([D, GCOL], F32, tag="z",
                                  name=f"z_{layer}_{g}")
                    nc.tensor.matmul(zp[:, :w], lhsT=wl_sb, rhs=aggT[:, :w],
                                     start=True, stop=False)
                    nc.tensor.matmul(zp[:, :w], lhsT=wr_sb,
                                     rhs=own_sb[:, b0 * P:b0 * P + w],
                                     start=False, stop=True)
                    if layer == 0:
                        outT = hT_sb
                        nc.scalar.activation(out=hT_sb[:, b0 * P:b0 * P + w],
                                             in_=zp[:, :w], func=func,
                                             bias=bb_sb[:, 0:1], scale=1.0)
                    else:
                        outT = gpool.tile([D, GCOL], F32, tag="outT",
                                          name=f"oT_{rep}_{g}")
                        nc.scalar.activation(out=outT[:, :w], in_=zp[:, :w],
                                             func=func, bias=bb_sb[:, 0:1],
                                             scale=1.0)
                    if "store" not in parts:
                        continue
                    for bi in range(nb):
                        b = b0 + bi
                        tp = psS.tile([P, D],
                                      BF16 if layer == 0 else F32,
                                      tag="tr", name=f"tp_{layer}_{b}")
                        sl = (slice(b * P, b * P + P) if layer == 0
                              else slice(bi * P, bi * P + P))
                        nc.tensor.transpose(
                            out=tp, in_=outT[:, sl],
                            identity=ident_sb if layer == 0 else ident32_sb)
                        if layer == 0:
                            nc.scalar.copy(out=nodeh_sb[:, b * P:b * P + D],
                                           in_=tp)
                        else:
                            nc.scalar.copy(out=nodeo_sb[:, b * D:(b + 1) * D],
                                           in_=tp)

                if layer == 0 and "store" in parts:
                    # p-major store: h_chunk row p*NBLK+b = node b*128+p
                    nc.sync.dma_start(
                        out=h_chunk.rearrange("(p b) f -> p b f", b=NBLK),
                        in_=nodeh_sb.rearrange("p (b f) -> p b f", f=P),
                    )
                elif layer == 1 and "store" in parts:
                    nc.sync.dma_start(
                        out=out_dr.ap().rearrange("(b p) f -> p b f", p=P),
                        in_=nodeo_sb.rearrange("p (b f) -> p b f", f=D),
                    )
                if layer == 0 and "collective" in parts:
                    if one_core:
                        nc.sync.dma_start(out=h_full[0:NP_, :], in_=h_chunk)
                    else:
                        nc.gpsimd.collective_compute(
                            "AllGather",
                            mybir.AluOpType.bypass,
                            replica_groups=[list(range(NC))],
                            ins=[h_chunk.opt()],
                            outs=[h_full.opt()],
                        )
                    if use_tbl:
                        # reload the token table with h (contiguous)
                        nc.sync.dma_start(
                            out=tblh.rearrange("p (c b f) -> p c b f",
                                               c=NC, f=P),
                            in_=h_full.rearrange("(c p b) f -> p c b f",
                                                 p=P, b=NBLK),
                        )

    nc.compile()
    return nc


def make_in_maps(meta, x, W_l1, b_l1, W_r1, W_l2, b_l2, W_r2):
    x = np.ascontiguousarray(np.asarray(x, dtype=np.float32))
    xb = x.astype(BF)
    xn = np.zeros((N, P), BF)
    xn[:, :D] = xb
    # wrapped p-major copy: row p*NRANK_X + t = node t*128+p (padded)
    xpad = np.zeros((NTOK_X, P), BF)
    xpad[:N, :D] = xb
    xw = np.ascontiguousarray(
        xpad.reshape(NRANK_X, P, P).transpose(1, 0, 2).reshape(P * NRANK_X, P))
    iota = np.tile(np.arange(P, dtype=np.float32), (P, 1)).astype(BF)
    ident = np.eye(D, dtype=np.float32).astype(BF)
    common = {
        "xn": xn,
        "xw": xw,
        "wl1t": np.ascontiguousarray(np.asarray(W_l1, np.float32).T).astype(BF),
        "wr1t": np.ascontiguousarray(np.asarray(W_r1, np.float32).T).astype(BF),
        "wl2t": np.ascontiguousarray(np.asarray(W_l2, np.float32).T).astype(BF),
        "wr2t": np.ascontiguousarray(np.asarray(W_r2, np.float32).T).astype(BF),
        "b1": np.asarray(b_l1, np.float32).reshape(D, 1).copy(),
        "b2": np.asarray(b_l2, np.float32).reshape(D, 1).copy(),
        "iota": iota,
        "ident": ident,
        "ident32": np.eye(D, dtype=np.float32),
    }
    in_maps = []
    for k in range(NC):
        xo = xb[k * N_OWN:(k + 1) * N_OWN]
        xoT = np.zeros((D, NP_), BF)
        xoT[:, :N_OWN] = xo.T
        in_maps.append(dict(common, xoT=xoT, idx=meta.idx[k],
                            dstf=meta.dstf[k], invb=meta.invb[k]))
    return in_maps


_CACHE = {}
_LAST_RES = None


def kernel(x, edge_index, W_l1, b_l1, W_r1, W_l2, b_l2, W_r2):
    edge_index = np.asarray(edge_index)
    x = np.asarray(x)
    key = hash(edge_index.tobytes())
    if key in _CACHE:
        meta, nc = _CACHE[key]
    else:
        meta = preprocess(edge_index)
        nc = build_program(meta)
        _CACHE[key] = (meta, nc)
    in_maps = make_in_maps(meta, x, W_l1, b_l1, W_r1, W_l2, b_l2, W_r2)
    res = run_bass_kernel_spmd(nc, in_maps, core_ids=list(range(NC)))
    global _LAST_RES
    _LAST_RES = res
    out = np.concatenate(
        [res.results[k]["out"][:N_OWN] for k in range(NC)], axis=0
    )
    return out.astype(np.float32)
